# revision 16
# baseline (speedup 1.0000x reference)
"""Trainium2 Bass kernel for DirectionalConv2D (wind-directed 5x5 Gaussian blur).

Reference math (per pixel):
    theta = arctan2(v, u+1e-8);  c, s = cos(theta), sin(theta)
    w(dx,dy) = exp(-(dx*c + dy*s)^2 / 4.5)        for dx,dy in [-2..2]
    spread   = sum(w * fire[h+dx, w+dy]) / (sum(w) + 1e-8)   (zero padded)
    out      = clip(0.7*spread + 0.3*fire, 0, 1)

v5 design (v1 measured 40.1us), from NTFF traces of v1-v4:
  * One semaphore PER DMA TRANSFER: SWDGE/HWDGE queue transfers overlap,
    so shared counters raced (intermittent corruption in v1/v3).
  * NO compute on GpSimd/Pool (shared SBUF port collapses DVE ~4-20x).
  * Wind first: wu whole on sync HWDGE, wv whole on scalar HWDGE, both
    sem-visible ~10.4us; fire (ident/fA/fB) alone on the SWDGE queue.
  * ACT table load at block entry (dummy Exp with no waits).
  * Host stages wu*(4/3); uu via Square's free affine (scale=0.75);
    uv43=wu43*wv gives cs43=(4/3)cs with no extra op.
  * Column-split spine: uu/Ln/ir2 (ACT) and r2/ss (DVE) per half.
  * MERGED exps (pairs sharing scale+bias as ONE FD=2048 ACT op over
    co-allocated sources [ss|cc], [cs43|-cs43], [m12|m1m2]):
      E1 [w01|w10] = exp(-k*[ss|cc])       E2 [w02|w20] = exp(-4k*.)
      E3 [w11|w1m1]= exp(-1.5k*[cs|-cs]-k) E4 [w22|w2m2]= exp(-6k*.-4k)
      E5 [w12|w1m2]= exp(-3k*[m12|m1m2]-k)
      E6 [w2m1|w21]= exp(+3k*[m12|m1m2]-4k) split per 512-col bank.
    q = 8(ss-1/2)^2 = Square(s8*ss - s8/2) also on ACT; ser on DVE.
  * Grouped pairsums: |dy|-partner pairs differ by a constant offset ->
    5 of 6 slot-pairs are ONE 4D-AP tensor_tensor each.
  * MAC groups issued AS SOON as their exps land (A-gated), pairsums
    fill the gaps -> the PE is fed early.
  * BATCHED PE matmuls: zero-stride output AP accumulates a [128,n,512]
    rhs (n product slots) onto one PSUM bank in ONE matmul, so the PE
    runs 10 matmul instructions instead of 26.
  * NO clip: spread,fire in [0,1] -> 0.7*spread+0.3*fire in [0,1] up to
    ~1e-3 rounding (tol 2e-2); final per-bank op writes cen03+tfull
    straight to fp32; out DMAs issue from sync (bank0) / scalar (bank1).
  * fp16 everywhere on the fire/weight path; fA + fB (one-column-shifted
    copy) keep all 25 tap views 4B-aligned for DVE 2x_1p mode.
  * 1/r2 via Exp(-Ln(r2 + 1e-4)); ACT->ACT RAW self-fenced via A.
  * Raw bass: monotone per-engine semaphore thresholds, standalone waits.
"""

import sys

if "/opt/trn_rl_repo" not in sys.path:
    sys.path.insert(0, "/opt/trn_rl_repo")

import numpy as np

B, H, W = 4, 512, 512
N_CORES = 8
HS = H // 2
K = 1.0 / 4.5
C0 = 0.040093331769199714
C1 = 0.0007997721694363273

_NC = None

# weight slot order (= w_all / pst / prod slot order), merged-exp pairs
# adjacent; PE batches: slots 0-3, 4-7, 8-9, 10-11 (last per bank)
PAIR_ORDER = [
    (0, 1), (1, 0),                    # E1
    (0, 2), (2, 0),                    # E2
    (1, 1), (1, -1),                   # E3
    (2, 2), (2, -2),                   # E4
    (1, 2), (1, -2),                   # E5
    (2, -1), (2, 1),                   # E6 (bank-split)
]
SLOT = {p: i for i, p in enumerate(PAIR_ORDER)}


def _build_nc():
    import math

    import concourse.bass as bass
    import concourse.mybir as mybir
    from concourse.ap import AP

    dt = mybir.dt
    AF = mybir.ActivationFunctionType
    OP = mybir.AluOpType
    f16 = dt.float16
    f32 = dt.float32
    k = K
    s8 = math.sqrt(8.0)

    nc = bass.Bass(detect_race_conditions=False)

    # ---- DRAM io ----
    fA_d = nc.dram_tensor("fA", [128, 6, 516], f16, kind="ExternalInput")
    fB_d = nc.dram_tensor("fB", [128, 6, 516], f16, kind="ExternalInput")
    # per partition: [wu*(4/3) (1024) | wv (1024)]
    wuv_d = nc.dram_tensor("wuv", [128, 2048], f16, kind="ExternalInput")
    id_d = nc.dram_tensor("ident", [128, 128], f16, kind="ExternalInput")
    out_d = nc.dram_tensor("out", [128, 1024], f32, kind="ExternalOutput")

    def sb(name, shape, dtype=f16):
        return nc.alloc_sbuf_tensor(name, shape, dtype).ap()

    fA = sb("fA_t", [128, 6, 516])
    fB = sb("fB_t", [128, 6, 516])
    wuv = sb("wuv_t", [128, 2048])
    wu43 = wuv[:, 0:1024]
    wv = wuv[:, 1024:2048]
    ident = sb("id_t", [128, 128])
    uu = sb("uu", [128, 1024])
    vv = sb("vv", [128, 1024])
    uv43 = sb("uv43", [128, 1024])
    r2 = sb("r2", [128, 1024])
    lnr = sb("lnr", [128, 1024], f32)
    ir2 = sb("ir2", [128, 1024])
    # merged-exp source pairs live in one tensor each
    sscc = sb("sscc", [128, 2048])
    ss = sscc[:, 0:1024]
    cc = sscc[:, 1024:2048]
    cs2 = sb("cs2", [128, 2048])
    cs43 = cs2[:, 0:1024]
    cs43m = cs2[:, 1024:2048]
    mm2 = sb("mm2", [128, 2048])
    m12 = mm2[:, 0:1024]
    m1m2 = mm2[:, 1024:2048]
    q = sb("q", [128, 1024])
    ser = sb("ser", [128, 1024])
    w_all = sb("w_all", [128, 12, 1024])
    pst = sb("pst", [128, 12, 1024])
    prod = sb("prod", [128, 12, 1024])
    tfull = sb("tfull", [128, 1024])
    cen03 = sb("cen03", [128, 1024])
    outt = sb("outt", [128, 1024], f32)
    dummy = sb("dummy_t", [128, 1], f32)
    dummy_in = sb("dummy_in", [128, 1], f32)

    acc = nc.alloc_psum_tensor("acc", [128, 1024], f32).ap()

    def V(dx, dy, half=None):
        # tap view [128, 2, 512] (or [128, 512] row `half`); fB holds the
        # one-column-shifted copy so odd dy keeps 4B-aligned starts
        if dy % 2 == 0:
            t, c0 = fA, 2 + dy
        else:
            t, c0 = fB, 1 + dy
        if half is None:
            return t[:, 2 + dx : 4 + dx, c0 : c0 + 512]
        return t[:, 2 + dx + half, c0 : c0 + 512]

    bias_vals = sorted({0.0, 1e-4, -k, -4 * k, -s8 / 2, C0 - C1})

    with (
        nc.semaphore("swu") as WU,
        nc.semaphore("swu2") as WU2,
        nc.semaphore("swv") as WV,
        nc.semaphore("sfi") as FI,
        nc.semaphore("sfa") as FAA,
        nc.semaphore("sfb") as FB,
        nc.semaphore("sa") as A,
        nc.semaphore("sv") as Vs,
        nc.semaphore("sp") as P,
        nc.semaphore("sy") as SY,
        nc.semaphore("sb") as SB,
    ):
        # ---- pre-block: Ln-bias const + whole wu/wv on the HWDGE queues ----
        cb = {}
        for bi_i, val in enumerate(bias_vals):
            if (f32, val) in nc.const_aps.aps:
                continue
            t = nc.alloc_sbuf_tensor(f"constb{bi_i}", [128, 1], f32)
            cb[val] = t.ap()
            nc.const_aps.aps[(f32, val)] = t.ap()
        nc.gpsimd.memset(cb[1e-4], 1e-4).then_inc(SB, 1)

        # wu halves: per-transfer sems keep this safe even if the queue
        # interleaves; the first half unblocks uu0 ~0.4us earlier
        nc.sync.dma_start(wuv[:, 0:512], wuv_d[:, 0:512]).then_inc(WU, 16)
        nc.sync.dma_start(wuv[:, 512:1024], wuv_d[:, 512:1024]).then_inc(
            WU2, 16
        )
        nc.scalar.dma_start(wuv[:, 1024:2048], wuv_d[:, 1024:2048]).then_inc(
            WV, 16
        )

        with nc.Block() as block:

            @block.gpsimd
            def _(gpsimd):
                # fire alone on the SWDGE queue, one sem per transfer; fA
                # first so the DVE's fill pairsum can run during the ir2
                # wait; fire tensors move as one 6192B descriptor/partition
                gpsimd.dma_start(fA[:, :, :], fA_d[:, :, :]).then_inc(FAA, 16)
                gpsimd.dma_start(ident, id_d[:, :]).then_inc(FI, 16)
                gpsimd.dma_start(fB[:, :, :], fB_d[:, :, :]).then_inc(FB, 16)
                for val in (-k, -4 * k, -s8 / 2, C0 - C1):
                    gpsimd.memset(cb[val], val)
                gpsimd.memset(dummy_in, 0.0).then_inc(SB, 1)  # SB=2

            @block.sync
            def _(sync):
                sync.wait_ge(Vs, 11)  # out bank 0 ready
                sync.dma_start(out_d[:, 0:512], outt[:, 0:512]).then_inc(SY, 16)

            @block.scalar
            def _(scalar):
                a_count = [0]

                def aop(emit):
                    emit().then_inc(A, 1)
                    a_count[0] += 1

                # no waits: the block's single ACT_TABLE_LOAD (inserted
                # before this) runs immediately, during the wind-DMA wait
                aop(lambda: scalar.activation(dummy, dummy_in, AF.Exp))         # A1
                # host staged wu*(4/3); Square's free affine un-scales it
                scalar.wait_ge(WU, 16)
                aop(lambda: scalar.activation(uu[:, 0:512], wu43[:, 0:512],
                                              AF.Square, scale=0.75))           # A2
                scalar.wait_ge(WU2, 16)
                aop(lambda: scalar.activation(uu[:, 512:1024],
                                              wu43[:, 512:1024],
                                              AF.Square, scale=0.75))           # A3
                scalar.wait_ge(SB, 1)
                # +1e-4 bias keeps ir2 <= 1e4 (fp16-safe) with no clamp op
                scalar.wait_ge(Vs, 1)
                aop(lambda: scalar.activation(lnr[:, 0:512], r2[:, 0:512],
                                              AF.Ln, bias=1e-4))                # A4
                scalar.wait_ge(Vs, 2)
                aop(lambda: scalar.activation(lnr[:, 512:1024],
                                              r2[:, 512:1024],
                                              AF.Ln, bias=1e-4))                # A5
                scalar.wait_ge(A, 5)    # self-fence: ACT->ACT RAW on lnr
                aop(lambda: scalar.activation(ir2[:, 0:512], lnr[:, 0:512],
                                              AF.Exp, scale=-1.0))              # A6
                scalar.wait_ge(A, 6)    # keep the pipe drained for half 1
                aop(lambda: scalar.activation(ir2[:, 512:1024],
                                              lnr[:, 512:1024],
                                              AF.Exp, scale=-1.0))              # A7

                def wexp(lo, src2, sc, bi):
                    # one FD=2048 exp -> w_all slots lo, lo+1
                    aop(lambda: scalar.activation(
                        w_all[:, lo : lo + 2, :],
                        src2.rearrange("p (s x) -> p s x", s=2),
                        AF.Exp, bias=bi, scale=sc))

                scalar.wait_ge(SB, 2)
                scalar.wait_ge(Vs, 3)   # cc (implies ss)
                wexp(0, sscc, -k, 0.0)                                          # A8
                wexp(2, sscc, -4 * k, 0.0)                                      # A9
                scalar.wait_ge(Vs, 4)   # cs43m (implies cs43)
                wexp(4, cs2, -1.5 * k, -k)                                      # A10
                wexp(6, cs2, -6 * k, -4 * k)                                    # A11
                scalar.wait_ge(Vs, 5)   # m1m2 (implies m12)
                wexp(8, mm2, -3 * k, -k)                                        # A12
                # E6 split per 512-col bank: earlier stop for bank 0
                mm2v = mm2.rearrange("p (s x) -> p s x", s=2)
                aop(lambda: scalar.activation(
                    w_all[:, 10:12, 0:512], mm2v[:, :, 0:512],
                    AF.Exp, bias=-4 * k, scale=3 * k))                          # A13
                aop(lambda: scalar.activation(
                    w_all[:, 10:12, 512:1024], mm2v[:, :, 512:1024],
                    AF.Exp, bias=-4 * k, scale=3 * k))                          # A14
                # q = (s8*ss - s8/2)^2 = 1 + cos(4*theta)
                aop(lambda: scalar.activation(q, ss, AF.Square,
                                              bias=-s8 / 2, scale=s8))          # A15
                scalar.wait_ge(A, 15)   # self-fence: ACT->ACT RAW on q
                aop(lambda: scalar.activation(ser, q, AF.Identity,
                                              bias=C0 - C1, scale=C1))          # A16
                scalar.wait_ge(FAA, 16)  # long satisfied; hygiene
                aop(lambda: scalar.activation(
                    cen03.rearrange("p (a b) -> p a b", a=2), V(0, 0),
                    AF.Identity, bias=0.0, scale=0.3))                          # A17
                assert a_count[0] == 17
                # out-DMA bank 1 issues from here (scalar is idle now)
                scalar.wait_ge(Vs, 12)
                scalar.dma_start(out_d[:, 512:1024], outt[:, 512:1024]).then_inc(
                    SY, 16
                )

            @block.vector
            def _(vector):
                def ps1(slot):
                    p = PAIR_ORDER[slot]
                    vector.tensor_tensor(
                        pst[:, slot, :].rearrange("p (a b) -> p a b", a=2),
                        V(*p), V(-p[0], -p[1]), OP.add)

                def ps2(slot):
                    # grouped pairsum for slots slot, slot+1 (|dy| partners
                    # differ by a constant offset -> one 4D-AP TT)
                    pa, pb = PAIR_ORDER[slot], PAIR_ORDER[slot + 1]
                    va, vb = V(*pa), V(*pb)
                    na = V(-pa[0], -pa[1])
                    nb = V(-pb[0], -pb[1])
                    src0 = AP(tensor=va.tensor, offset=va.offset,
                              ap=[list(va.ap[0]),
                                  [vb.offset - va.offset, 2]]
                              + [list(x) for x in va.ap[1:]])
                    src1 = AP(tensor=na.tensor, offset=na.offset,
                              ap=[list(na.ap[0]),
                                  [nb.offset - na.offset, 2]]
                              + [list(x) for x in na.ap[1:]])
                    dst = pst[:, slot : slot + 2, :].rearrange(
                        "p s (a b) -> p s a b", a=2)
                    vector.tensor_tensor(dst, src0, src1, OP.add)

                def mac_of(lo, hi):
                    sl = slice(lo, hi)
                    vector.tensor_tensor(
                        prod[:, sl, :], w_all[:, sl, :], pst[:, sl, :], OP.mult
                    ).then_inc(Vs, 1)

                vector.wait_ge(WV, 16)
                vector.tensor_tensor(vv, wv, wv, OP.mult)
                vector.wait_ge(A, 2)
                vector.tensor_tensor(r2[:, 0:512], uu[:, 0:512], vv[:, 0:512],
                                     OP.add).then_inc(Vs, 1)                    # Vs1
                vector.wait_ge(A, 3)
                vector.tensor_tensor(r2[:, 512:1024], uu[:, 512:1024],
                                     vv[:, 512:1024],
                                     OP.add).then_inc(Vs, 1)                    # Vs2
                vector.tensor_tensor(uv43, wu43, wv, OP.mult)
                # fA's pairsum fills the Ln/Exp wait window (fA is the
                # FIRST transfer on the SWDGE queue so it lands by ~13us)
                vector.wait_ge(FAA, 16)
                ps1(1)          # (1,0)
                # spine next: ss/cc gate the exps
                vector.wait_ge(A, 6)
                vector.tensor_tensor(ss[:, 0:512], vv[:, 0:512],
                                     ir2[:, 0:512], OP.mult)
                vector.tensor_scalar(out=cc[:, 0:512], in0=ss[:, 0:512],
                                     scalar1=-1.0, scalar2=1.0,
                                     op0=OP.mult, op1=OP.add)
                vector.wait_ge(A, 7)
                vector.tensor_tensor(ss[:, 512:1024], vv[:, 512:1024],
                                     ir2[:, 512:1024], OP.mult)
                vector.tensor_scalar(out=cc[:, 512:1024], in0=ss[:, 512:1024],
                                     scalar1=-1.0, scalar2=1.0,
                                     op0=OP.mult, op1=OP.add).then_inc(Vs, 1)   # Vs3
                vector.tensor_tensor(cs43, uv43, ir2, OP.mult)
                vector.tensor_scalar(out=cs43m, in0=cs43, scalar1=-1.0,
                                     scalar2=0.0, op0=OP.mult,
                                     op1=OP.add).then_inc(Vs, 1)                # Vs4
                vector.tensor_tensor(m12, ss, cs43, OP.add)
                vector.tensor_tensor(m1m2, ss, cs43,
                                     OP.subtract).then_inc(Vs, 1)               # Vs5
                # fire-gated fills + A-gated MAC groups ASAP
                ps2(2)          # (0,2),(2,0)
                vector.wait_ge(FB, 16)
                ps1(0)          # (0,1)
                vector.wait_ge(A, 9)
                mac_of(0, 4)                                                    # Vs6
                ps2(4)          # (1,1),(1,-1)
                ps2(6)          # (2,2),(2,-2)
                vector.wait_ge(A, 11)
                mac_of(4, 8)                                                    # Vs7
                ps2(8)          # (1,2),(1,-2)
                ps2(10)         # (2,-1),(2,1)
                vector.wait_ge(A, 12)
                mac_of(8, 10)                                                   # Vs8
                vector.wait_ge(A, 13)
                vector.tensor_tensor(
                    prod[:, 10:12, 0:512], w_all[:, 10:12, 0:512],
                    pst[:, 10:12, 0:512], OP.mult).then_inc(Vs, 1)              # Vs9
                vector.wait_ge(A, 14)
                vector.tensor_tensor(
                    prod[:, 10:12, 512:1024], w_all[:, 10:12, 512:1024],
                    pst[:, 10:12, 512:1024], OP.mult).then_inc(Vs, 1)           # Vs10
                # finale per PSUM bank: (acc*ser + cen03) -> fp32, no clip
                vector.wait_ge(A, 16)   # ser (ACT)
                vector.wait_ge(P, 1)
                vector.tensor_tensor(tfull[:, 0:512], acc[:, 0:512],
                                     ser[:, 0:512], OP.mult)
                vector.wait_ge(A, 17)   # cen03 (ACT)
                vector.tensor_tensor(outt[:, 0:512], cen03[:, 0:512],
                                     tfull[:, 0:512], OP.add).then_inc(Vs, 1)   # Vs11
                vector.wait_ge(P, 2)
                vector.tensor_tensor(tfull[:, 512:1024], acc[:, 512:1024],
                                     ser[:, 512:1024], OP.mult)
                vector.tensor_tensor(outt[:, 512:1024], cen03[:, 512:1024],
                                     tfull[:, 512:1024],
                                     OP.add).then_inc(Vs, 1)                    # Vs12

            @block.tensor
            def _(tensor):
                def mm(h, rhs, start, stop=False):
                    return tensor.matmul(out=acc[:, 512 * h : 512 * h + 512],
                                         lhsT=ident, rhs=rhs,
                                         start=start, stop=stop,
                                         skip_group_check=True)

                tensor.wait_ge(FI, 16)
                tensor.wait_ge(FAA, 16)
                mm(0, V(0, 0, 0), True)      # center, row 0
                mm(1, V(0, 0, 1), True)      # center, row 1
                for g, (lo, hi) in enumerate(((0, 4), (4, 8), (8, 10))):
                    tensor.wait_ge(Vs, 6 + g)
                    for i in range(lo, hi):
                        mm(0, prod[:, i, 0:512], False)
                        mm(1, prod[:, i, 512:1024], False)
                tensor.wait_ge(Vs, 9)
                mm(0, prod[:, 10, 0:512], False)
                mm(0, prod[:, 11, 0:512], False, True).then_inc(P, 1)
                tensor.wait_ge(Vs, 10)
                mm(1, prod[:, 10, 512:1024], False)
                mm(1, prod[:, 11, 512:1024], False, True).then_inc(P, 1)

    return nc


def _get_nc():
    global _NC
    if _NC is None:
        _NC = _build_nc()
    return _NC


def _make_in_maps(fire_map, wind_u, wind_v):
    from numpy.lib.stride_tricks import sliding_window_view

    ident = np.eye(128, dtype=np.float16)
    in_maps = []
    for b in range(B):
        fp = np.pad(np.asarray(fire_map[b, 0], np.float32), ((2, 2), (2, 3)))
        fp16 = fp.astype(np.float16)  # [516, 517]
        for t in range(2):
            shard = fp16[t * HS : t * HS + HS + 4]  # [260, 517]
            swv = sliding_window_view(shard, (6, 516))  # [255, 2, 6, 516]
            fA = np.ascontiguousarray(swv[::2, 0])
            fB = np.ascontiguousarray(swv[::2, 1])
            wus = ((4.0 / 3.0) * np.asarray(
                wind_u[b, 0, t * HS : (t + 1) * HS], np.float32)
                   ).reshape(128, 1024).astype(np.float16)
            wvs = (np.asarray(wind_v[b, 0, t * HS : (t + 1) * HS], np.float32)
                   .reshape(128, 1024).astype(np.float16))
            wuv = np.ascontiguousarray(np.concatenate([wus, wvs], axis=1))
            in_maps.append(
                {"fA": fA, "fB": fB, "wuv": wuv, "ident": ident})
    return in_maps


def _gather(results):
    out = np.empty((B, 1, H, W), np.float32)
    for ci, r in enumerate(results):
        b, t = divmod(ci, 2)
        out[b, 0, t * HS : (t + 1) * HS] = r["out"].reshape(HS, W)
    return out


def _run(fire_map, wind_u, wind_v, trace=False):
    from concourse.bass_utils import run_bass_kernel_spmd

    in_maps = _make_in_maps(fire_map, wind_u, wind_v)
    res = run_bass_kernel_spmd(_get_nc(), in_maps, list(range(N_CORES)), trace=trace)
    return _gather(res.results), res


def kernel(fire_map, wind_u, wind_v):
    out, _ = _run(fire_map, wind_u, wind_v, trace=False)
    return out
